# revision 30
# baseline (speedup 1.0000x reference)
"""Trainium2 Bass kernel for AdvancedHomeostaticCell.

Math (per batch row x of D=128, weights [128,128] except Wf [128,256]):
    i = sigmoid(x@Wi.T + bi)
    f = sigmoid(x@Wfx.T + cf)            # cf = Wf_b + hp@Wfh.T folded (hp const)
    c = x@Wc.T + bc                      # Wc = Wslow + Wfast combined
    h = i*c + f*hp
    o = sigmoid(h@Wo.T + bo)
    h_out = o*tanh(h)
    out = (h_out - mean)*rsqrt(var+eps)*g + b    # layernorm

Device layout: feature-on-partition end to end, zero transposes.  The gate
matmuls stream x^T (pre-transposed on host); per-feature biases ride as
per-partition STT scalars / rank-1 matmuls; Wo streams h directly.  LN
statistics (row sums of h_out and h_out^2) are computed on the PE by
accumulating ones-block matmuls into one [16, 512] PSUM tile per 8-chunk
group — each chunk's sums land on their own psum partition row, so the
reduction costs only 2 extra 512-col streams per chunk and two tiny
stationaries.  The device ships h_out (feature-major bf16) plus the raw
sum/sumsq rows; the host applies the final per-row scale-shift
(x-mu)*rsqrt(var+eps) fused with the ln_g/ln_b affine during the f32 upcast
+ detranspose it performs anyway.  Input/output move as ~1MB slab DMAs with
1-8KB contiguous lines per partition.

Sharding: pure data-parallel over batch across 8 NeuronCores (SPMD).
"""

import numpy as np
import ml_dtypes

D = 128
B_FULL = 262144
NCORES = 8
B_LOC = B_FULL // NCORES        # 32768 rows per core
CHUNK = 512                     # batch rows per chunk
GROUP = 8                       # chunks per slab/stats group
N_CHUNK = B_LOC // CHUNK        # 64
N_GROUP = N_CHUNK // GROUP      # 8
EPS = 1e-5

_CACHE = {}


def _build(nzb=(False, False, False)):
    from contextlib import ExitStack
    import concourse.bass as bass
    import concourse.tile as tile
    from concourse import bacc, mybir

    F32 = mybir.dt.float32
    BF16 = mybir.dt.bfloat16
    AF = mybir.ActivationFunctionType
    OP = mybir.AluOpType

    NZ_BI, NZ_BO, NZ_BC = nzb
    SLAB = GROUP * CHUNK        # 4096 batch cols per slab

    nc = bacc.Bacc("TRN2", target_bir_lowering=False, debug=False,
                   num_devices=NCORES)

    xT_d = nc.dram_tensor("xT", [D, B_LOC], BF16, kind="ExternalInput").ap()
    wg_d = nc.dram_tensor("wg", [4 * D, D], BF16, kind="ExternalInput").ap()
    gb_d = nc.dram_tensor("gbias", [1, 3 * D], BF16,
                          kind="ExternalInput").ap()
    pc_d = nc.dram_tensor("pcol", [D, 4], F32, kind="ExternalInput").ap()
    ob_d = nc.dram_tensor("oblk", [D, 2 * GROUP * 2 * GROUP], BF16,
                          kind="ExternalInput").ap()
    out_d = nc.dram_tensor("out", [D, B_LOC], BF16, kind="ExternalOutput").ap()
    st_d = nc.dram_tensor("stats", [2 * GROUP, N_GROUP, CHUNK], F32,
                          kind="ExternalOutput").ap()

    with tile.TileContext(nc) as tc, ExitStack() as ctx:
        const = ctx.enter_context(tc.tile_pool(name="const", bufs=1))
        xsl = ctx.enter_context(tc.tile_pool(name="xsl", bufs=2))
        gp = ctx.enter_context(tc.tile_pool(name="gp", bufs=5))
        hp_ = ctx.enter_context(tc.tile_pool(name="hp_", bufs=5))
        ps_if = ctx.enter_context(tc.tile_pool(name="ps_if", bufs=2,
                                               space="PSUM"))
        ps_c = ctx.enter_context(tc.tile_pool(name="ps_c", bufs=2,
                                              space="PSUM"))
        ps_o = ctx.enter_context(tc.tile_pool(name="ps_o", bufs=1,
                                              space="PSUM"))
        ps_s = ctx.enter_context(tc.tile_pool(name="ps_s", bufs=1,
                                              space="PSUM"))

        # --- constants -----------------------------------------------------
        w_i = const.tile([D, D], BF16, tag="w_i")
        w_f = const.tile([D, D], BF16, tag="w_f")
        w_c = const.tile([D, D], BF16, tag="w_c")
        w_o = const.tile([D, D], BF16, tag="w_o")
        gbias = const.tile([1, 3, D], BF16, tag="gbias")   # bi, cf, bo rows
        pcol = const.tile([D, 4], F32, tag="pcol")     # (hp, bc, bi, cf)
        oblk = const.tile([D, 2 * GROUP, 2 * GROUP], BF16,
                          tag="oblk")  # ones blocks
        ones_row = const.tile([1, CHUNK], BF16, tag="ones_row")
        for k, w in enumerate((w_i, w_f, w_c, w_o)):
            nc.sync.dma_start(w[:], wg_d[k * D:(k + 1) * D, :])
        nc.sync.dma_start(gbias[:], gb_d.rearrange("o (k d) -> o k d", k=3))
        nc.sync.dma_start(pcol[:], pc_d[:, :])
        nc.sync.dma_start(oblk[:], ob_d.rearrange("p (r m) -> p r m",
                                                  m=2 * GROUP))
        nc.gpsimd.memset(ones_row[:], 1.0)
        hp_ap = pcol[:, 0:1]
        bc_ap = pcol[:, 1:2]
        bi_ap = pcol[:, 2:3]
        cf_ap = pcol[:, 3:4]

        # Software-pipelined emission: per iteration c, each engine queue
        # receives work whose inputs were produced 1-4 iterations earlier, so
        # in-order engine FIFOs never block on a late dependency and the PE
        # stays HAM-warm.
        #   PE:  A(c) gates     D(c-2) Wo      H(c-4) stats
        #   ACT: E1(c-2) tanh   B(c) sig_if    E2(c-2) sig_o
        #   DVE: C(c-1) gating  G(c-3) sq
        #   GPS: F(c-2) product
        xs_ts = {}
        S_ts = {}
        tl = {}

        def load_slab(g):
            if g < N_GROUP and g not in xs_ts:
                xs_slab = xsl.tile([D, SLAB], BF16, tag="xs")
                nc.sync.dma_start(xs_slab[:],
                                  xT_d[:, g * SLAB:(g + 1) * SLAB])
                xs_ts[g] = xs_slab

        load_slab(0)
        PIPE = 6
        for c in range(N_CHUNK + PIPE):
            g, s = divmod(c, GROUP)
            if c < N_CHUNK:
                if s == 0:
                    load_slab(g + 1)
                # --- A(c): gate matmuls ---------------------------------
                xs = xs_ts[g][:, s * CHUNK:(s + 1) * CHUNK]
                p1 = ps_if.tile([D, 2, CHUNK], F32, tag="p1")
                pc = ps_c.tile([D, CHUNK], F32, tag="pc")
                nc.tensor.matmul(p1[:, 0, :], w_i[:], xs,
                                 start=True, stop=not NZ_BI)
                if NZ_BI:
                    nc.tensor.matmul(p1[:, 0, :], gbias[:, 0, :],
                                     ones_row[:], start=False, stop=True)
                nc.tensor.matmul(p1[:, 1, :], w_f[:], xs,
                                 start=True, stop=False)
                nc.tensor.matmul(p1[:, 1, :], gbias[:, 1, :], ones_row[:],
                                 start=False, stop=True)
                nc.tensor.matmul(pc[:], w_c[:], xs)
                tl[c] = {"p1": p1, "pc": pc}

            d = c - 3
            if 0 <= d < N_CHUNK:
                # --- D(d): Wo matmul ------------------------------------
                po = ps_o.tile([D, CHUNK], F32, tag="po")
                nc.tensor.matmul(po[:], w_o[:], tl[d]["h"][:],
                                 start=True, stop=not NZ_BO)
                if NZ_BO:
                    nc.tensor.matmul(po[:], gbias[:, 2, :], ones_row[:],
                                     start=False, stop=True)
                tl[d]["po"] = po

            hh = c - PIPE
            if 0 <= hh < N_CHUNK:
                # --- H(hh): stats matmuls into S ------------------------
                gh, sh = divmod(hh, GROUP)
                if sh == 0:
                    S_g = ps_s.tile([2 * GROUP, CHUNK], F32, tag="S")
                    S_ts[gh] = S_g
                S = S_ts[gh]
                nc.tensor.matmul(S[:, :], oblk[:, 2 * sh, :],
                                 tl[hh]["hout"][:], start=(sh == 0),
                                 stop=False, skip_group_check=True)
                nc.tensor.matmul(S[:, :], oblk[:, 2 * sh + 1, :],
                                 tl[hh]["sq"][:], start=False,
                                 stop=(sh == GROUP - 1),
                                 skip_group_check=True)
                nc.sync.dma_start(
                    out_d[:, hh * CHUNK:(hh + 1) * CHUNK], tl[hh]["hout"][:])
                if sh == GROUP - 1:
                    S_sb = gp.tile([2 * GROUP, CHUNK], F32, tag="S_sb")
                    nc.vector.tensor_copy(S_sb[:], S[:])
                    nc.sync.dma_start(st_d[:, gh, :], S_sb[:])

            if 0 <= d < N_CHUNK:
                # --- E1(d): tanh(h) -------------------------------------
                tanh_t = gp.tile([D, CHUNK], BF16, tag="tanh_t")
                nc.scalar.activation(tanh_t[:], tl[d]["h"][:], AF.Tanh)
                tl[d]["tanh_t"] = tanh_t

            if c < N_CHUNK:
                # --- B(c): sigmoid(i|f) ---------------------------------
                ift = gp.tile([D, 2, CHUNK], BF16, tag="ift")
                nc.scalar.activation(ift[:], tl[c]["p1"][:], AF.Sigmoid)
                tl[c]["ift"] = ift

            if 0 <= d < N_CHUNK:
                # --- E2(d): sigmoid(o) ----------------------------------
                o_t = gp.tile([D, CHUNK], BF16, tag="o_t")
                nc.scalar.activation(o_t[:], tl[d]["po"][:], AF.Sigmoid)
                tl[d]["o_t"] = o_t

            b = c - 1
            if 0 <= b < N_CHUNK:
                # --- C(b): gating on DVE --------------------------------
                ift = tl[b]["ift"]
                t1 = gp.tile([D, CHUNK], BF16, tag="t1")
                if NZ_BC:
                    nc.vector.scalar_tensor_tensor(
                        t1[:], tl[b]["pc"][:], bc_ap, ift[:, 0, :],
                        OP.add, OP.mult)
                else:
                    nc.vector.tensor_tensor(t1[:], tl[b]["pc"][:],
                                            ift[:, 0, :], OP.mult)
                h = gp.tile([D, CHUNK], BF16, tag="h")
                nc.vector.scalar_tensor_tensor(
                    h[:], ift[:, 1, :], hp_ap, t1[:], OP.mult, OP.add)
                tl[b]["h"] = h

            e = c - 4
            if 0 <= e < N_CHUNK:
                # --- G(e): square on DVE --------------------------------
                sq = gp.tile([D, CHUNK], BF16, tag="sq")
                nc.vector.tensor_tensor(sq[:], tl[e]["hout"][:],
                                        tl[e]["hout"][:], OP.mult)
                tl[e]["sq"] = sq

            if 0 <= d < N_CHUNK:
                # --- F(d): h_out product (DVE) --------------------------
                hout = hp_.tile([D, CHUNK], BF16, tag="hout")
                nc.vector.tensor_tensor(hout[:], tl[d]["o_t"][:],
                                        tl[d]["tanh_t"][:], OP.mult)
                tl[d]["hout"] = hout

            if hh - 1 >= 0 and (hh - 1) in tl:
                del tl[hh - 1]

    nc.compile()
    return nc


def _prep_host(inputs):
    BF = ml_dtypes.bfloat16
    x = np.asarray(inputs["x"], dtype=np.float32)
    hp = np.asarray(inputs["h_prev"], dtype=np.float32)[0]          # [128]
    Wf = np.asarray(inputs["Wf_w"], dtype=np.float32)
    W_comb = (np.asarray(inputs["W_slow_w"], dtype=np.float32)
              + np.asarray(inputs["W_fast_w"], dtype=np.float32))
    wg = np.concatenate([
        np.asarray(inputs["Wi_w"], dtype=np.float32).T,
        Wf[:, :D].T,
        W_comb.T,
        np.asarray(inputs["Wo_w"], dtype=np.float32).T,
    ], axis=0).astype(BF)                                           # [4D, D]
    bi = np.asarray(inputs["Wi_b"], dtype=np.float32)
    cf = np.asarray(inputs["Wf_b"], dtype=np.float32) + hp @ Wf[:, D:].T
    bo = np.asarray(inputs["Wo_b"], dtype=np.float32)
    bc = np.asarray(inputs["W_slow_b"], dtype=np.float32)
    gbias = np.concatenate([bi, cf, bo]).astype(BF).reshape(1, 3 * D)
    pcol = np.stack([hp, bc, bi, cf], axis=1).astype(np.float32)    # [D, 4]
    xT = np.asarray(x.reshape(NCORES, B_LOC, D).transpose(0, 2, 1),
                    order="C").astype(BF)                           # [n,D,B]
    nzb = (bool(np.any(bi)), bool(np.any(bo)), bool(np.any(bc)))
    return xT, wg, gbias, pcol, nzb


def _make_oblk():
    # 16 stationary blocks, each [D, 16] bf16: block r has ones in column r.
    BF = ml_dtypes.bfloat16
    ob = np.zeros((D, 2 * GROUP, 2 * GROUP), np.float32)
    for r in range(2 * GROUP):
        ob[:, r, r] = 1.0
    return ob.astype(BF).reshape(D, 2 * GROUP * 2 * GROUP)


def kernel(**inputs):
    from concourse.bass_utils import run_bass_kernel_spmd

    xT, wg, gbias, pcol, nzb = _prep_host(inputs)
    oblk = _make_oblk()
    key = ("nc", nzb)
    if key not in _CACHE:
        _CACHE[key] = _build(nzb=nzb)
    nc = _CACHE[key]

    in_maps = [
        {"xT": np.ascontiguousarray(xT[i]), "wg": wg, "gbias": gbias,
         "pcol": pcol, "oblk": oblk}
        for i in range(NCORES)
    ]
    import os
    trace = bool(os.environ.get("BASS_TRACE"))
    rr = run_bass_kernel_spmd(nc, in_maps, list(range(NCORES)), trace=trace)
    _CACHE["last_rr"] = rr

    ln_g = np.asarray(inputs["ln_g"], dtype=np.float32)
    ln_b = np.asarray(inputs["ln_b"], dtype=np.float32)
    parts = []
    for i in range(NCORES):
        hout = np.asarray(rr.results[i]["out"]).astype(np.float32)
        st = np.asarray(rr.results[i]["stats"])    # [16, N_GROUP, 512] f32
        # row 2c+j of group g covers batch rows g*4096 + c*512 + [0,512)
        s1 = st[0::2, :, :].transpose(1, 0, 2).reshape(B_LOC)   # g, c, b
        s2 = st[1::2, :, :].transpose(1, 0, 2).reshape(B_LOC)
        mu = s1 / D
        var = s2 / D - mu * mu
        r = 1.0 / np.sqrt(var + EPS)
        # hout is [D, B_LOC] feature-major; fuse detranspose + scale-shift
        outp = (hout.T - mu[:, None]) * r[:, None]
        outp = outp * ln_g + ln_b
        parts.append(outp)
    out = np.concatenate(parts, axis=0)
    return out.astype(np.float32)


# revision 31
# speedup vs baseline: 1.1953x; 1.1953x over previous
"""Trainium2 Bass kernel for AdvancedHomeostaticCell.

Math (per batch row x of D=128, weights [128,128] except Wf [128,256]):
    i = sigmoid(x@Wi.T + bi)
    f = sigmoid(x@Wfx.T + cf)            # cf = Wf_b + hp@Wfh.T folded (hp const)
    c = x@Wc.T + bc                      # Wc = Wslow + Wfast combined
    h = i*c + f*hp
    o = sigmoid(h@Wo.T + bo)
    h_out = o*tanh(h)
    out = (h_out - mean)*rsqrt(var+eps)*g + b    # layernorm

Device layout: feature-on-partition end to end, zero transposes.  The gate
matmuls stream x^T (pre-transposed on host); per-feature biases ride as
per-partition STT scalars / rank-1 matmuls; Wo streams h directly.  LN
statistics (row sums of h_out and h_out^2) are computed on the PE by
accumulating ones-block matmuls into one [16, 512] PSUM tile per 8-chunk
group — each chunk's sums land on their own psum partition row, so the
reduction costs only 2 extra 512-col streams per chunk and two tiny
stationaries.  The device ships h_out (feature-major bf16) plus the raw
sum/sumsq rows; the host applies the final per-row scale-shift
(x-mu)*rsqrt(var+eps) fused with the ln_g/ln_b affine during the f32 upcast
+ detranspose it performs anyway.  Input/output move as ~1MB slab DMAs with
1-8KB contiguous lines per partition.

Sharding: pure data-parallel over batch across 8 NeuronCores (SPMD).
"""

import numpy as np
import ml_dtypes

D = 128
B_FULL = 262144
NCORES = 8
B_LOC = B_FULL // NCORES        # 32768 rows per core
CHUNK = 512                     # batch rows per chunk
GROUP = 8                       # chunks per slab/stats group
N_CHUNK = B_LOC // CHUNK        # 64
N_GROUP = N_CHUNK // GROUP      # 8
EPS = 1e-5

_CACHE = {}


def _build(nzb=(False, False, False)):
    from contextlib import ExitStack
    import concourse.bass as bass
    import concourse.tile as tile
    from concourse import bacc, mybir

    F32 = mybir.dt.float32
    BF16 = mybir.dt.bfloat16
    AF = mybir.ActivationFunctionType
    OP = mybir.AluOpType

    NZ_BI, NZ_BO, NZ_BC = nzb
    SLAB = GROUP * CHUNK        # 4096 batch cols per slab

    nc = bacc.Bacc("TRN2", target_bir_lowering=False, debug=False,
                   num_devices=NCORES)

    xT_d = nc.dram_tensor("xT", [D, B_LOC], BF16, kind="ExternalInput").ap()
    wg_d = nc.dram_tensor("wg", [4 * D, D], BF16, kind="ExternalInput").ap()
    gb_d = nc.dram_tensor("gbias", [1, 3 * D], BF16,
                          kind="ExternalInput").ap()
    pc_d = nc.dram_tensor("pcol", [D, 4], F32, kind="ExternalInput").ap()
    ob_d = nc.dram_tensor("oblk", [D, 2 * GROUP * 2 * GROUP], BF16,
                          kind="ExternalInput").ap()
    out_d = nc.dram_tensor("out", [D, B_LOC], BF16, kind="ExternalOutput").ap()
    st_d = nc.dram_tensor("stats", [2 * GROUP, N_GROUP, CHUNK], F32,
                          kind="ExternalOutput").ap()

    with tile.TileContext(nc) as tc, ExitStack() as ctx:
        const = ctx.enter_context(tc.tile_pool(name="const", bufs=1))
        xsl = ctx.enter_context(tc.tile_pool(name="xsl", bufs=2))
        gp = ctx.enter_context(tc.tile_pool(name="gp", bufs=5))
        hp_ = ctx.enter_context(tc.tile_pool(name="hp_", bufs=5))
        ps_if = ctx.enter_context(tc.tile_pool(name="ps_if", bufs=2,
                                               space="PSUM"))
        ps_c = ctx.enter_context(tc.tile_pool(name="ps_c", bufs=2,
                                              space="PSUM"))
        ps_o = ctx.enter_context(tc.tile_pool(name="ps_o", bufs=1,
                                              space="PSUM"))
        ps_s = ctx.enter_context(tc.tile_pool(name="ps_s", bufs=1,
                                              space="PSUM"))

        # --- constants -----------------------------------------------------
        w_i = const.tile([D, D], BF16, tag="w_i")
        w_f = const.tile([D, D], BF16, tag="w_f")
        w_c = const.tile([D, D], BF16, tag="w_c")
        w_o = const.tile([D, D], BF16, tag="w_o")
        gbias = const.tile([1, 3, D], BF16, tag="gbias")   # bi, cf, bo rows
        pcol = const.tile([D, 4], F32, tag="pcol")     # (hp, bc, bi, cf)
        oblk = const.tile([D, 2 * GROUP, 2 * GROUP], BF16,
                          tag="oblk")  # ones blocks
        ones_row = const.tile([1, CHUNK], BF16, tag="ones_row")
        for k, w in enumerate((w_i, w_f, w_c, w_o)):
            nc.sync.dma_start(w[:], wg_d[k * D:(k + 1) * D, :])
        nc.sync.dma_start(gbias[:], gb_d.rearrange("o (k d) -> o k d", k=3))
        nc.sync.dma_start(pcol[:], pc_d[:, :])
        nc.sync.dma_start(oblk[:], ob_d.rearrange("p (r m) -> p r m",
                                                  m=2 * GROUP))
        nc.gpsimd.memset(ones_row[:], 1.0)
        hp_ap = pcol[:, 0:1]
        bc_ap = pcol[:, 1:2]
        bi_ap = pcol[:, 2:3]
        cf_ap = pcol[:, 3:4]

        # Software-pipelined emission: per iteration c, each engine queue
        # receives work whose inputs were produced 1-4 iterations earlier, so
        # in-order engine FIFOs never block on a late dependency and the PE
        # stays HAM-warm.
        #   PE:  A(c) gates     D(c-2) Wo      H(c-4) stats
        #   ACT: E1(c-2) tanh   B(c) sig_if    E2(c-2) sig_o
        #   DVE: C(c-1) gating  G(c-3) sq
        #   GPS: F(c-2) product
        xs_ts = {}
        S_ts = {}
        tl = {}

        def load_slab(g):
            if g < N_GROUP and g not in xs_ts:
                xs_slab = xsl.tile([D, SLAB], BF16, tag="xs")
                nc.sync.dma_start(xs_slab[:],
                                  xT_d[:, g * SLAB:(g + 1) * SLAB])
                xs_ts[g] = xs_slab

        load_slab(0)
        PIPE = 4
        for c in range(N_CHUNK + PIPE):
            g, s = divmod(c, GROUP)
            if c < N_CHUNK:
                if s == 0:
                    load_slab(g + 1)
                # --- A(c): gate matmuls ---------------------------------
                xs = xs_ts[g][:, s * CHUNK:(s + 1) * CHUNK]
                p1 = ps_if.tile([D, 2, CHUNK], F32, tag="p1")
                pc = ps_c.tile([D, CHUNK], F32, tag="pc")
                nc.tensor.matmul(p1[:, 0, :], w_i[:], xs,
                                 start=True, stop=not NZ_BI)
                if NZ_BI:
                    nc.tensor.matmul(p1[:, 0, :], gbias[:, 0, :],
                                     ones_row[:], start=False, stop=True)
                nc.tensor.matmul(p1[:, 1, :], w_f[:], xs,
                                 start=True, stop=False)
                nc.tensor.matmul(p1[:, 1, :], gbias[:, 1, :], ones_row[:],
                                 start=False, stop=True)
                nc.tensor.matmul(pc[:], w_c[:], xs)
                tl[c] = {"p1": p1, "pc": pc}

            d = c - 2
            if 0 <= d < N_CHUNK:
                # --- D(d): Wo matmul ------------------------------------
                po = ps_o.tile([D, CHUNK], F32, tag="po")
                nc.tensor.matmul(po[:], w_o[:], tl[d]["h"][:],
                                 start=True, stop=not NZ_BO)
                if NZ_BO:
                    nc.tensor.matmul(po[:], gbias[:, 2, :], ones_row[:],
                                     start=False, stop=True)
                tl[d]["po"] = po

            hh = c - PIPE
            if 0 <= hh < N_CHUNK:
                # --- H(hh): stats matmuls into S ------------------------
                gh, sh = divmod(hh, GROUP)
                if sh == 0:
                    S_g = ps_s.tile([2 * GROUP, CHUNK], F32, tag="S")
                    S_ts[gh] = S_g
                S = S_ts[gh]
                nc.tensor.matmul(S[:, :], oblk[:, 2 * sh, :],
                                 tl[hh]["hout"][:], start=(sh == 0),
                                 stop=False, skip_group_check=True)
                nc.tensor.matmul(S[:, :], oblk[:, 2 * sh + 1, :],
                                 tl[hh]["sq"][:], start=False,
                                 stop=(sh == GROUP - 1),
                                 skip_group_check=True)
                nc.sync.dma_start(
                    out_d[:, hh * CHUNK:(hh + 1) * CHUNK], tl[hh]["hout"][:])
                if sh == GROUP - 1:
                    S_sb = gp.tile([2 * GROUP, CHUNK], F32, tag="S_sb")
                    nc.vector.tensor_copy(S_sb[:], S[:])
                    nc.sync.dma_start(st_d[:, gh, :], S_sb[:])

            if 0 <= d < N_CHUNK:
                # --- E1(d): tanh(h) -------------------------------------
                tanh_t = gp.tile([D, CHUNK], BF16, tag="tanh_t")
                nc.scalar.activation(tanh_t[:], tl[d]["h"][:], AF.Tanh)
                tl[d]["tanh_t"] = tanh_t

            if c < N_CHUNK:
                # --- B(c): sigmoid(i|f) ---------------------------------
                ift = gp.tile([D, 2, CHUNK], BF16, tag="ift")
                nc.scalar.activation(ift[:], tl[c]["p1"][:], AF.Sigmoid)
                tl[c]["ift"] = ift

            if 0 <= d < N_CHUNK:
                # --- E2(d): sigmoid(o) ----------------------------------
                o_t = gp.tile([D, CHUNK], BF16, tag="o_t")
                nc.scalar.activation(o_t[:], tl[d]["po"][:], AF.Sigmoid)
                tl[d]["o_t"] = o_t

            b = c - 1
            if 0 <= b < N_CHUNK:
                # --- C(b): gating on DVE --------------------------------
                ift = tl[b]["ift"]
                t1 = gp.tile([D, CHUNK], BF16, tag="t1")
                if NZ_BC:
                    nc.vector.scalar_tensor_tensor(
                        t1[:], tl[b]["pc"][:], bc_ap, ift[:, 0, :],
                        OP.add, OP.mult)
                else:
                    nc.vector.tensor_tensor(t1[:], tl[b]["pc"][:],
                                            ift[:, 0, :], OP.mult)
                h = gp.tile([D, CHUNK], BF16, tag="h")
                nc.vector.scalar_tensor_tensor(
                    h[:], ift[:, 1, :], hp_ap, t1[:], OP.mult, OP.add)
                tl[b]["h"] = h

            e = c - 3
            if 0 <= e < N_CHUNK:
                # --- G(e): square on DVE --------------------------------
                sq = gp.tile([D, CHUNK], BF16, tag="sq")
                nc.vector.tensor_tensor(sq[:], tl[e]["hout"][:],
                                        tl[e]["hout"][:], OP.mult)
                tl[e]["sq"] = sq

            if 0 <= d < N_CHUNK:
                # --- F(d): h_out product (DVE) --------------------------
                hout = hp_.tile([D, CHUNK], BF16, tag="hout")
                nc.vector.tensor_tensor(hout[:], tl[d]["o_t"][:],
                                        tl[d]["tanh_t"][:], OP.mult)
                tl[d]["hout"] = hout

            if hh - 1 >= 0 and (hh - 1) in tl:
                del tl[hh - 1]

    nc.compile()
    return nc


def _prep_host(inputs):
    BF = ml_dtypes.bfloat16
    x = np.asarray(inputs["x"], dtype=np.float32)
    hp = np.asarray(inputs["h_prev"], dtype=np.float32)[0]          # [128]
    Wf = np.asarray(inputs["Wf_w"], dtype=np.float32)
    W_comb = (np.asarray(inputs["W_slow_w"], dtype=np.float32)
              + np.asarray(inputs["W_fast_w"], dtype=np.float32))
    wg = np.concatenate([
        np.asarray(inputs["Wi_w"], dtype=np.float32).T,
        Wf[:, :D].T,
        W_comb.T,
        np.asarray(inputs["Wo_w"], dtype=np.float32).T,
    ], axis=0).astype(BF)                                           # [4D, D]
    bi = np.asarray(inputs["Wi_b"], dtype=np.float32)
    cf = np.asarray(inputs["Wf_b"], dtype=np.float32) + hp @ Wf[:, D:].T
    bo = np.asarray(inputs["Wo_b"], dtype=np.float32)
    bc = np.asarray(inputs["W_slow_b"], dtype=np.float32)
    gbias = np.concatenate([bi, cf, bo]).astype(BF).reshape(1, 3 * D)
    pcol = np.stack([hp, bc, bi, cf], axis=1).astype(np.float32)    # [D, 4]
    xT = np.asarray(x.reshape(NCORES, B_LOC, D).transpose(0, 2, 1),
                    order="C").astype(BF)                           # [n,D,B]
    nzb = (bool(np.any(bi)), bool(np.any(bo)), bool(np.any(bc)))
    return xT, wg, gbias, pcol, nzb


def _make_oblk():
    # 16 stationary blocks, each [D, 16] bf16: block r has ones in column r.
    BF = ml_dtypes.bfloat16
    ob = np.zeros((D, 2 * GROUP, 2 * GROUP), np.float32)
    for r in range(2 * GROUP):
        ob[:, r, r] = 1.0
    return ob.astype(BF).reshape(D, 2 * GROUP * 2 * GROUP)


def kernel(**inputs):
    from concourse.bass_utils import run_bass_kernel_spmd

    xT, wg, gbias, pcol, nzb = _prep_host(inputs)
    oblk = _make_oblk()
    key = ("nc", nzb)
    if key not in _CACHE:
        _CACHE[key] = _build(nzb=nzb)
    nc = _CACHE[key]

    in_maps = [
        {"xT": np.ascontiguousarray(xT[i]), "wg": wg, "gbias": gbias,
         "pcol": pcol, "oblk": oblk}
        for i in range(NCORES)
    ]
    import os
    trace = bool(os.environ.get("BASS_TRACE"))
    rr = run_bass_kernel_spmd(nc, in_maps, list(range(NCORES)), trace=trace)
    _CACHE["last_rr"] = rr

    ln_g = np.asarray(inputs["ln_g"], dtype=np.float32)
    ln_b = np.asarray(inputs["ln_b"], dtype=np.float32)
    parts = []
    for i in range(NCORES):
        hout = np.asarray(rr.results[i]["out"]).astype(np.float32)
        st = np.asarray(rr.results[i]["stats"])    # [16, N_GROUP, 512] f32
        # row 2c+j of group g covers batch rows g*4096 + c*512 + [0,512)
        s1 = st[0::2, :, :].transpose(1, 0, 2).reshape(B_LOC)   # g, c, b
        s2 = st[1::2, :, :].transpose(1, 0, 2).reshape(B_LOC)
        mu = s1 / D
        var = s2 / D - mu * mu
        r = 1.0 / np.sqrt(var + EPS)
        # hout is [D, B_LOC] feature-major; fuse detranspose + scale-shift
        outp = (hout.T - mu[:, None]) * r[:, None]
        outp = outp * ln_g + ln_b
        parts.append(outp)
    out = np.concatenate(parts, axis=0)
    return out.astype(np.float32)
